# revision 1
# baseline (speedup 1.0000x reference)
"""DualMemorySystem Trainium2 kernel — 8-core SPMD (batch x 4 row-bands).

Per core: one (b, 32-row out band). Convolution form of unfold/attention/fold:
  sim = conv(x, mem)      -> p matmuls per branch, K=(kernel-row i, c), fp32r
  att = softmax_m(sim)    -> exp (ACT), ones-matmul partition sum,
                             reciprocal_approx (DVE) + multiply (GpSimd)
  R_i = conv_x(att, mem)  -> matmuls over col-shifted att replicas, fp16
  out = fold_y(R)         -> log-tree of shifted adds; partition-group moves
                             via SBUF->SBUF DMA (engines are lane-aligned)
  fusion: pooled partials + 4-core AllReduce + tiny MLP + weighted combine.

Hardware constraints baked in (probed): matmul dst partition base must be 0;
engines cannot remap partitions (only DMA/PE move data across partitions);
no divide ALU on DVE; fp32r matmul needs N>=256; DVE ops need 32-aligned
partition bases.
"""
import numpy as np
from contextlib import ExitStack

import concourse.bass as bass
import concourse.bacc as bacc
import concourse.tile as tile
from concourse import mybir
from concourse.bass_utils import run_bass_kernel_spmd

F32 = mybir.dt.float32
F32R = mybir.dt.float32r
F16 = mybir.dt.float16

B, C, H, W = 2, 16, 128, 128
PS = (3, 5, 7)
PADS = (1, 2, 3)
NBG, NTG = 64, 8
NCORES = 8
NBANDS = 4
BH = H // NBANDS            # 32 out rows per core
RX = 38                     # x replica rows per core
CX = 134                    # x cols with halo (128 + 6)
RA = 38                     # max att rows (32 + 2*padmax)
RAL = 40                    # R sbuf rows (fold-tree halo)
RAS = [BH + 2 * p for p in PADS]   # att rows per branch: 34, 36, 38

_CACHE = {}


def _build_program(repeat=1, collective=True, groups8=False, phase="mono"):
    nc = bacc.Bacc("TRN2", target_bir_lowering=False, debug=False,
                   num_devices=NCORES)

    # ---------------- DRAM I/O ----------------
    if phase != "B":
        d_x8bg = nc.dram_tensor("x8bg", [128, RX, CX], F32, kind="ExternalInput")
        d_x8tg = nc.dram_tensor("x8tg", [128, RX, CX], F32, kind="ExternalInput")
        d_hug = nc.dram_tensor("hug", [3, RA, W], F32, kind="ExternalInput")
        d_rdiv = nc.dram_tensor("rdiv", [96, BH, W], F16, kind="ExternalInput")
        d_ones = nc.dram_tensor("oneslhs", [73, 72], F32, kind="ExternalInput")
        d_w1 = {}
        for s, M in (("bg", NBG), ("tg", NTG)):
            for n, p in enumerate(PS):
                d_w1[(s, n)] = nc.dram_tensor(
                    f"w1{s}{n}", [16 * p, p, M], F32, kind="ExternalInput")
        d_w2bg = nc.dram_tensor("w2bg", [128, 9, 128], F16, kind="ExternalInput")
        d_w2tg = nc.dram_tensor("w2tg", [64, 3, 128], F16, kind="ExternalInput")
    d_mw1 = nc.dram_tensor("mlpw1t", [80, 8], F32, kind="ExternalInput")
    d_mb1 = nc.dram_tensor("mlpb1", [4, 2], F32, kind="ExternalInput")
    d_mw2 = nc.dram_tensor("mlpw2t", [4, 96], F32, kind="ExternalInput")
    d_mb2 = nc.dram_tensor("mlpb2", [48, 2], F32, kind="ExternalInput")
    d_obg = nc.dram_tensor("out_bg", [C, BH, W], F16, kind="ExternalOutput")
    d_otg = nc.dram_tensor("out_tg", [C, BH, W], F16, kind="ExternalOutput")
    d_ccin = nc.dram_tensor("ccin", [32], F32)
    d_ccout = nc.dram_tensor("ccout", [32], F32)
    d_fdiv = d_pool = d_poolsum = None
    if phase == "A":
        d_fdiv = nc.dram_tensor("fdiv_out", [96, BH, W], F16,
                                kind="ExternalOutput")
        d_pool = nc.dram_tensor("pool_out", [32], F32, kind="ExternalOutput")
    if phase == "B":
        d_fdiv = nc.dram_tensor("fdiv_in", [96, BH, W], F16,
                                kind="ExternalInput")
        d_poolsum = nc.dram_tensor("poolsum", [32], F32, kind="ExternalInput")

    groups = [[0, 1, 2, 3, 4, 5, 6, 7]] if groups8 else [[0, 1, 2, 3], [4, 5, 6, 7]]

    with tile.TileContext(nc) as tc, ExitStack() as ctx:
        P = ctx.enter_context(tc.tile_pool(name="persist", bufs=1))
        pE = ctx.enter_context(tc.tile_pool(name="epool", bufs=1))
        pEtg = ctx.enter_context(tc.tile_pool(name="etg", bufs=2))
        pRcp = ctx.enter_context(tc.tile_pool(name="rcp", bufs=2))
        pSbg = ctx.enter_context(tc.tile_pool(name="sbg", bufs=1))
        pStg = ctx.enter_context(tc.tile_pool(name="stg", bufs=1))
        pR = ctx.enter_context(tc.tile_pool(name="rsb", bufs=2))
        pAB = ctx.enter_context(tc.tile_pool(name="foldtmp", bufs=1))
        psum_bg = ctx.enter_context(
            tc.tile_pool(name="psbg", bufs=2, space=bass.MemorySpace.PSUM))
        psum_tg = ctx.enter_context(
            tc.tile_pool(name="pstg", bufs=1, space=bass.MemorySpace.PSUM))
        psum_mix = ctx.enter_context(
            tc.tile_pool(name="pmix", bufs=2, space=bass.MemorySpace.PSUM))

        # body as closure so timing builds can repeat it
        def _body():
            # ---------------- persistent loads ----------------
            if phase != "B":
                x8 = {}
                for s, d in (("bg", d_x8bg), ("tg", d_x8tg)):
                    t = P.tile([128, RX, CX], F32R, tag=f"x8{s}")
                    nc.gpsimd.dma_start(t[:], d[:])
                    x8[s] = t
                rdiv = P.tile([96, BH, W], F16, tag="rdiv")
                nc.sync.dma_start(rdiv[:], d_rdiv[:])
                ones_l = P.tile([73, 72], F32R, tag="ones")
                nc.gpsimd.dma_start(ones_l[:], d_ones[:])
                w1 = {}
                for s, M in (("bg", NBG), ("tg", NTG)):
                    for n, p in enumerate(PS):
                        t = P.tile([16 * p, p, M], F32R, tag=f"w1{s}{n}")
                        nc.gpsimd.dma_start(t[:], d_w1[(s, n)][:])
                        w1[(s, n)] = t
                w2bg = P.tile([128, 9, 128], F16, tag="w2bg")
                nc.sync.dma_start(w2bg[:], d_w2bg[:])
                w2tg = P.tile([64, 3, 128], F16, tag="w2tg")
                nc.sync.dma_start(w2tg[:], d_w2tg[:])
            mw1 = P.tile([80, 8], F32, tag="mw1")
            nc.sync.dma_start(mw1[:], d_mw1[:])
            mb1 = P.tile([4, 2], F32, tag="mb1")
            nc.sync.dma_start(mb1[:], d_mb1[:])
            mw2 = P.tile([4, 96], F32, tag="mw2")
            nc.sync.dma_start(mw2[:], d_mw2[:])
            mb2 = P.tile([48, 2], F32, tag="mb2")
            nc.sync.dma_start(mb2[:], d_mb2[:])

            feats = P.tile([96, BH, W], F16, tag="feats")
            fdiv = P.tile([96, BH, W], F16, tag="fdiv")
            pacc = P.tile([96, 1], F32, tag="pacc")

            w2bg_base = [0, 2, 5]

            # ---------------- branches ----------------
            for n, p in enumerate(PS) if phase != "B" else []:
                pad = PADS[n]
                Ra = RAS[n]
                rxo = 6 - 2 * pad           # x8 row offset for this branch

                E = pE.tile([73, RA, W], F32R, tag="E")
                nc.gpsimd.dma_start(E[72:73, :, :], d_hug[n:n + 1, :, :])

                Sbg = pSbg.tile([128, RA, 136], F16, tag="Sbg")
                Stg = pStg.tile([72, RA, 144], F16, tag="Stg")
                nc.gpsimd.memset(Sbg[:, :, 0:4], 0.0)
                nc.gpsimd.memset(Sbg[:, :, 131:136], 0.0)
                nc.gpsimd.memset(Stg[:, :, 0:15], 0.0)
                nc.gpsimd.memset(Stg[:, :, 136:144], 0.0)

                # --- conv1 bg (4-row windows) + exp -> E[0:64] ---
                nwin = (Ra + 3) // 4
                for wnd in range(nwin):
                    r0 = 4 * wnd
                    rr = min(4, Ra - r0)
                    st = psum_bg.tile([64, 4, W], F32, tag="simbg")
                    for j in range(p):
                        nc.tensor.matmul(
                            st[0:64, 0:rr, :],
                            w1[("bg", n)][:, j, 0:64],
                            x8["bg"][0:16 * p, r0 + rxo:r0 + rxo + rr,
                                     j + 3 - pad:j + 3 - pad + W],
                            start=(j == 0), stop=(j == p - 1))
                    nc.scalar.activation(E[0:64, r0:r0 + rr, :], st[0:64, 0:rr, :],
                                         mybir.ActivationFunctionType.Exp)

                # --- conv1 tg (8-row tiles) + exp + DMA into E[64:72] ---
                nch = (Ra + 7) // 8
                for ch in range(nch):
                    r0 = 8 * ch
                    rr = min(8, Ra - r0)
                    stg = psum_tg.tile([8, 8, W], F32, tag="simtg")
                    for h in range(0, rr, 4):
                        hh = min(4, rr - h)
                        for j in range(p):
                            nc.tensor.matmul(
                                stg[0:8, h:h + hh, :],
                                w1[("tg", n)][:, j, :],
                                x8["tg"][0:16 * p, r0 + h + rxo:r0 + h + rxo + hh,
                                         j + 3 - pad:j + 3 - pad + W],
                                start=(j == 0), stop=(j == p - 1))
                    etg = pEtg.tile([8, 8, W], F32R, tag="etg")
                    nc.scalar.activation(etg[0:8, 0:rr, :], stg[0:8, 0:rr, :],
                                         mybir.ActivationFunctionType.Exp)
                    nc.sync.dma_start(E[64:72, r0:r0 + rr, :], etg[0:8, 0:rr, :])

                # --- softmax denom, reciprocal, att multiplies, replicas ---
                for ch in range(nch):
                    r0 = 8 * ch
                    rr = min(8, Ra - r0)
                    sb = psum_mix.tile([128, 8, W], F32, tag="mix")
                    for h in range(0, rr, 4):
                        hh = min(4, rr - h)
                        nc.tensor.matmul(
                            sb[0:72, h:h + hh, :],
                            ones_l[:, 0:72],
                            E[:, r0 + h:r0 + h + hh, :],
                            start=True, stop=True)
                    rcp = pRcp.tile([72, 8, W], F32, tag="rcp")
                    nc.vector.reciprocal_approx_fast(rcp[0:72, 0:rr, :],
                                                     sb[0:72, 0:rr, :])
                    nc.gpsimd.tensor_mul(Sbg[0:64, r0:r0 + rr, 3:131],
                                         E[0:64, r0:r0 + rr, :],
                                         rcp[0:64, 0:rr, :])
                    nc.gpsimd.tensor_mul(Stg[64:72, r0:r0 + rr, 8:136],
                                         E[64:72, r0:r0 + rr, :],
                                         rcp[64:72, 0:rr, :])
                    nc.sync.dma_start(Sbg[64:128, r0:r0 + rr, 4:132],
                                      Sbg[0:64, r0:r0 + rr, 3:131])
                    for g in range(p):
                        nc.sync.dma_start(Stg[8 * g:8 * g + 8, r0:r0 + rr,
                                              8 + g:136 + g],
                                          Stg[64:72, r0:r0 + rr, 8:136])

                # --- conv2: R matmuls + evac + fold, bg then tg ---
                for s in ("bg", "tg"):
                    Rsb = pR.tile([128, RAL, W], F16, tag="R")
                    nc.gpsimd.memset(Rsb[:, Ra:RAL, :], 0.0)
                    ntile = (Ra + 7) // 8
                    for tl in range(ntile):
                        r0 = 8 * tl
                        rr = min(8, Ra - r0)
                        rp = psum_mix.tile([128, 8, W], F32, tag="mix")
                        for h in range(0, rr, 4):
                            hh = min(4, rr - h)
                            if s == "bg":
                                nchk = (p + 1) // 2
                                for ci in range(nchk):
                                    jj = 2 * ci
                                    nc.tensor.matmul(
                                        rp[:, h:h + hh, :],
                                        w2bg[:, w2bg_base[n] + ci, :],
                                        Sbg[:, r0 + h:r0 + h + hh,
                                            3 + pad - jj:3 + pad - jj + W],
                                        start=(ci == 0), stop=(ci == nchk - 1))
                            else:
                                nc.tensor.matmul(
                                    rp[0:128, h:h + hh, :],
                                    w2tg[0:8 * p, n, :],
                                    Stg[0:8 * p, r0 + h:r0 + h + hh,
                                        8 + pad:8 + pad + W],
                                    start=True, stop=True)
                        # evac: ACT for bg, DVE for tg (balance engines)
                        if s == "bg":
                            nc.scalar.activation(Rsb[:, r0:r0 + rr, :],
                                                 rp[:, 0:rr, :],
                                                 mybir.ActivationFunctionType.Copy)
                        else:
                            nc.vector.tensor_copy(Rsb[:, r0:r0 + rr, :],
                                                  rp[:, 0:rr, :])

                    # fold tree: shifts via SBUF->SBUF DMA, lane-aligned adds
                    si = 0 if s == "bg" else 1
                    ftmp = pAB.tile([16, BH, W], F16, tag="Rs1")
                    if p == 3:
                        Rs1 = pAB.tile([64, 35, W], F16, tag="S1")
                        nc.sync.dma_start(Rs1[0:32, 0:33, :], Rsb[32:64, 2:35, :])
                        A = pAB.tile([64, 35, W], F16, tag="A")
                        nc.vector.tensor_add(A[0:32, 0:33, :], Rsb[0:32, 0:33, :],
                                             Rs1[0:32, 0:33, :])
                        As1 = pAB.tile([16, 33, W], F16, tag="B")
                        nc.sync.dma_start(As1[0:16, 0:BH, :], A[16:32, 1:BH + 1, :])
                        nc.vector.tensor_add(ftmp[:], A[0:16, 0:BH, :],
                                             As1[0:16, 0:BH, :])
                    else:
                        Rs1 = pAB.tile([64, 35, W], F16, tag="S1")
                        nc.sync.dma_start(Rs1[:, 0:35, :], Rsb[64:128, 4:39, :])
                        A = pAB.tile([64, 35, W], F16, tag="A")
                        nc.vector.tensor_add(A[:, 0:35, :], Rsb[0:64, 0:35, :],
                                             Rs1[:, 0:35, :])
                        As1 = pAB.tile([32, 33, W], F16, tag="S1")
                        nc.sync.dma_start(As1[0:32, 0:33, :], A[32:64, 2:35, :])
                        Bt = pAB.tile([32, 33, W], F16, tag="B")
                        nc.vector.tensor_add(Bt[:, 0:33, :], A[0:32, 0:33, :],
                                             As1[0:32, 0:33, :])
                        Bs1 = pAB.tile([16, 33, W], F16, tag="S1")
                        nc.sync.dma_start(Bs1[0:16, 0:BH, :], Bt[16:32, 1:BH + 1, :])
                        nc.vector.tensor_add(ftmp[:], Bt[0:16, 0:BH, :],
                                             Bs1[0:16, 0:BH, :])
                    nc.gpsimd.dma_start(
                        feats[32 * n + 16 * si:32 * n + 16 * si + 16, :, :],
                        ftmp[:])

            # ---------------- fusion ----------------
            if phase == "B":
                nc.sync.dma_start(fdiv[:], d_fdiv[:])
            if phase != "B":
              nc.vector.scalar_tensor_tensor(
                  fdiv[:], feats[:], 0.0, rdiv[:],
                  op0=mybir.AluOpType.bypass, op1=mybir.AluOpType.mult,
                  accum_out=pacc[:])
              pb = P.tile([32, 1], F32, tag="pb")
              pc = P.tile([32, 1], F32, tag="pc")
              nc.sync.dma_start(pb[:], pacc[32:64, :])
              nc.sync.dma_start(pc[:], pacc[64:96, :])
              pool32a = P.tile([32, 1], F32, tag="pool32a")
              pool32 = P.tile([32, 1], F32, tag="pool32")
              nc.vector.tensor_add(pool32a[:], pacc[0:32, :], pb[:])
              nc.vector.tensor_add(pool32[:], pool32a[:], pc[:])
            if phase == "A":
                nc.sync.dma_start(d_pool[:], pool32[:, 0])
                nc.sync.dma_start(d_fdiv[:], fdiv[:])
                return
            poolg = P.tile([80, 1], F32, tag="poolg")
            if phase == "mono":
                nc.sync.dma_start(d_ccin[:], pool32[:, 0])
                if collective:
                    nc.gpsimd.collective_compute(
                        "AllReduce", mybir.AluOpType.add, replica_groups=groups,
                        ins=[d_ccin[:]], outs=[d_ccout[:]])
                else:
                    nc.sync.dma_start(d_ccout[:], d_ccin[:])
                nc.sync.dma_start(poolg[0:16, 0], d_ccout[0:16])
                nc.sync.dma_start(poolg[64:80, 0], d_ccout[16:32])
            else:
                nc.sync.dma_start(poolg[0:16, 0], d_poolsum[0:16])
                nc.sync.dma_start(poolg[64:80, 0], d_poolsum[16:32])

            # tiny MLPs (fp32 matmuls; N=1)
            wt96 = P.tile([96, 1], F32, tag="wt96")
            for si in range(2):
                ph = psum_mix.tile([128, 8, W], F32, tag="mix")
                nc.tensor.matmul(ph[0:4, 0, 0:1],
                                 mw1[64 * si:64 * si + 16, 4 * si:4 * si + 4],
                                 poolg[64 * si:64 * si + 16, :],
                                 start=True, stop=True)
                hdn = P.tile([4, 1], F32, tag=f"hdn{si}")
                nc.scalar.activation(hdn[:], ph[0:4, 0, 0:1],
                                     mybir.ActivationFunctionType.Relu,
                                     bias=mb1[:, si:si + 1])
                ph2 = psum_mix.tile([128, 8, W], F32, tag="mix")
                nc.tensor.matmul(ph2[0:48, 0, 0:1], mw2[:, 48 * si:48 * si + 48],
                                 hdn[:], start=True, stop=True)
                elog = P.tile([48, 1], F32, tag=f"elog{si}")
                nc.scalar.activation(elog[:], ph2[0:48, 0, 0:1],
                                     mybir.ActivationFunctionType.Exp,
                                     bias=mb2[:, si:si + 1])
                e1 = P.tile([16, 1], F32, tag=f"e1{si}")
                e2 = P.tile([16, 1], F32, tag=f"e2{si}")
                nc.sync.dma_start(e1[:], elog[16:32, :])
                nc.sync.dma_start(e2[:], elog[32:48, :])
                s3a = P.tile([16, 1], F32, tag=f"s3a{si}")
                s3 = P.tile([16, 1], F32, tag=f"s3{si}")
                nc.vector.tensor_add(s3a[:], elog[0:16, :], e1[:])
                nc.vector.tensor_add(s3[:], s3a[:], e2[:])
                s3r = P.tile([16, 1], F32, tag=f"s3r{si}")
                nc.vector.reciprocal(s3r[:], s3[:])
                wdiv = P.tile([16, 3], F32, tag=f"wdiv{si}")
                nc.vector.tensor_mul(wdiv[:, 0:1], elog[0:16, :], s3r[:])
                nc.vector.tensor_mul(wdiv[:, 1:2], e1[:], s3r[:])
                nc.vector.tensor_mul(wdiv[:, 2:3], e2[:], s3r[:])
                for nn in range(3):
                    nc.sync.dma_start(
                        wt96[32 * nn + 16 * si:32 * nn + 16 * si + 16, :],
                        wdiv[:, nn:nn + 1])

            # weighted combine (lane-aligned; partition moves via DMA)
            nc.vector.tensor_scalar_mul(fdiv[:], fdiv[:], wt96[:])
            gb = pAB.tile([32, BH, W], F16, tag="S1")
            gc = pAB.tile([32, BH, W], F16, tag="A")
            nc.sync.dma_start(gb[:], fdiv[32:64, :, :])
            nc.gpsimd.dma_start(gc[:], fdiv[64:96, :, :])
            h32 = pAB.tile([32, BH, W], F16, tag="B")
            out32 = pAB.tile([32, BH, W], F16, tag="Rs1")
            nc.vector.tensor_add(h32[:], fdiv[0:32, :, :], gb[:])
            nc.vector.tensor_add(out32[:], h32[:], gc[:])
            nc.sync.dma_start(d_obg[:], out32[0:16, :, :])
            nc.sync.dma_start(d_otg[:], out32[16:32, :, :])

        for _rep in range(repeat):
            _body()

    nc.compile()
    return nc


# ======================= host-side prep =======================

def _prep_core(inputs, b, k):
    y0 = BH * k
    m = {}
    for s, key in (("bg", "bg"), ("tg", "tg")):
        x = np.asarray(inputs[key])[b]          # [C, H, W]
        x8 = np.zeros((8, C, RX, CX), np.float32)
        for g in range(8):
            lo = y0 - 6 + g
            hi = lo + RX
            slo, shi = max(lo, 0), min(hi, H)
            if slo < shi:
                x8[g, :, slo - lo:shi - lo, 3:131] = x[:, slo:shi, :]
        m[f"x8{s}"] = x8.reshape(128, RX, CX)

    hug = np.zeros((3, RA, W), np.float32)
    for n, pad in enumerate(PADS):
        for r in range(RA):
            y = y0 - pad + r
            if not (0 <= y < H):
                hug[n, r, :] = 1e30
    m["hug"] = hug

    rdiv = np.zeros((96, BH, W), np.float32)
    for n, pad in enumerate(PADS):
        yy = np.arange(H)
        rc = np.minimum(yy, pad) + np.minimum(H - 1 - yy, pad) + 1.0
        cc = np.minimum(yy[:W], pad) + np.minimum(W - 1 - yy[:W], pad) + 1.0
        div = np.outer(rc[y0:y0 + BH], cc) + 1e-8
        r = (1.0 / div).astype(np.float32)
        for si in range(2):
            base = 32 * n + 16 * si
            rdiv[base:base + 16] = r[None, :, :]
    m["rdiv"] = rdiv.astype(np.float16)

    ones = np.zeros((73, 72), np.float32)
    ones[0:64, 0:64] = 1.0
    ones[64:72, 64:72] = 1.0
    ones[72, :] = 1.0
    m["oneslhs"] = ones

    for s, M, nmem in (("bg", NBG, "bg_mem"), ("tg", NTG, "tg_mem")):
        for n, p in enumerate(PS):
            mem = np.asarray(inputs[f"{nmem}{n}"])          # [M, C*p*p]
            temp = float(np.asarray(inputs[f"{s}_temp{n}"])[0])
            D = C * p * p
            arr = mem.reshape(M, C, p, p)
            w1 = arr.transpose(2, 1, 3, 0).reshape(p * C, p, M).copy()
            m[f"w1{s}{n}"] = (w1 * (temp / np.sqrt(D))).astype(np.float32)

    # fold consumes group q at row shift +q where q = 2*pad - i
    w2bg = np.zeros((2, NBG, 9, 8, 16), np.float32)
    base = [0, 2, 5]
    for n, p in enumerate(PS):
        pad = PADS[n]
        arr = np.asarray(inputs[f"bg_mem{n}"]).reshape(NBG, C, p, p)
        for ci in range((p + 1) // 2):
            for g in range(2):
                j = 2 * ci + g
                if j < p:
                    for i in range(p):
                        w2bg[g, :, base[n] + ci, 2 * pad - i, :] = \
                            arr[:, :, i, j]
    m["w2bg"] = w2bg.reshape(128, 9, 128).astype(np.float16)

    w2tg = np.zeros((8, NTG, 3, 8, 16), np.float32)
    for n, p in enumerate(PS):
        pad = PADS[n]
        arr = np.asarray(inputs[f"tg_mem{n}"]).reshape(NTG, C, p, p)
        for g in range(p):
            for i in range(p):
                w2tg[g, :, n, 2 * pad - i, :] = arr[:, :, i, g]
    m["w2tg"] = w2tg.reshape(64, 3, 128).astype(np.float16)

    mw1 = np.zeros((80, 8), np.float32)
    mb1 = np.zeros((4, 2), np.float32)
    mw2 = np.zeros((4, 96), np.float32)
    mb2 = np.zeros((48, 2), np.float32)
    for si, s in enumerate(("bg", "tg")):
        mw1[64 * si:64 * si + 16, 4 * si:4 * si + 4] = (
            np.asarray(inputs[f"{s}_fc1_w"]).T / (H * W))
        mb1[:, si] = np.asarray(inputs[f"{s}_fc1_b"])
        mw2[:, 48 * si:48 * si + 48] = np.asarray(inputs[f"{s}_fc2_w"]).T
        mb2[:, si] = np.asarray(inputs[f"{s}_fc2_b"])
    m["mlpw1t"], m["mlpb1"], m["mlpw2t"], m["mlpb2"] = mw1, mb1, mw2, mb2
    return m


def kernel(**inputs):
    if "ncA" not in _CACHE:
        _CACHE["ncA"] = _build_program(phase="A")
        _CACHE["ncB"] = _build_program(phase="B")

    in_maps = []
    for core in range(NCORES):
        b, k = divmod(core, NBANDS)
        in_maps.append(_prep_core(inputs, b, k))

    resA = run_bass_kernel_spmd(_CACHE["ncA"], in_maps, list(range(NCORES)))

    # host glue: reduce pooled partials within each batch's 4-band group
    poolsum = {}
    for b in range(B):
        poolsum[b] = np.sum([resA.results[b * NBANDS + k]["pool_out"]
                             for k in range(NBANDS)], axis=0).astype(np.float32)

    in_mapsB = []
    for core in range(NCORES):
        b, k = divmod(core, NBANDS)
        mA = in_maps[core]
        in_mapsB.append({
            "fdiv_in": resA.results[core]["fdiv_out"],
            "poolsum": poolsum[b],
            "mlpw1t": mA["mlpw1t"], "mlpb1": mA["mlpb1"],
            "mlpw2t": mA["mlpw2t"], "mlpb2": mA["mlpb2"],
        })
    resB = run_bass_kernel_spmd(_CACHE["ncB"], in_mapsB, list(range(NCORES)))
    _CACHE["last_result"] = resB

    f_bc = np.zeros((B, C, H, W), np.float32)
    f_tg = np.zeros((B, C, H, W), np.float32)
    for core in range(NCORES):
        b, k = divmod(core, NBANDS)
        y0 = BH * k
        f_bc[b, :, y0:y0 + BH, :] = resB.results[core]["out_bg"].astype(np.float32)
        f_tg[b, :, y0:y0 + BH, :] = resB.results[core]["out_tg"].astype(np.float32)
    return (f_bc, f_tg)



# revision 5
# speedup vs baseline: 1.1166x; 1.1166x over previous
"""DualMemorySystem Trainium2 kernel — 8-core SPMD (batch x 4 row-bands).

Per core: one (b, 32-row out band). Convolution form of unfold/attention/fold:
  sim = conv(x, mem)      -> p matmuls per 4-row window, K=(kernel-row, c), fp32r
  att = softmax_m(sim)    -> exp (ACT), ones-matmul partition sum,
                             reciprocal_approx (DVE) + multiplies
  R_i = conv_x(att, mem)  -> matmuls over col-shifted att replicas, fp16
  out = fold_y(R)         -> log-tree of shifted adds (in-place), partition
                             moves via SBUF->SBUF DMA
  fusion: pooled partials via STT accum -> host MLP between launches ->
          tiny phase-B kernel applies softmax weights + combines.

Software-pipelined emission: branch order (p=7, p=5, p=3); per branch the
PE stream is conv1(k) [with denominator matmuls at lag-1], then conv2(k-1),
so the PE never waits on the current branch's softmax chain. Double-buffered
PSUM pools (8 banks exactly). DMAs with dependencies ride only the sync and
gpsimd queues so compute engines never stall on a DMA trigger.

Hardware constraints baked in (probed): matmul dst partition base must be 0;
engines cannot remap partitions (only DMA/PE move data across partitions);
no divide ALU on DVE; fp32r matmul needs N>=256; DVE ops need 32-aligned
partition bases.
"""
import numpy as np
from contextlib import ExitStack

import concourse.bass as bass
import concourse.bacc as bacc
import concourse.tile as tile
from concourse import mybir
from concourse.bass_utils import run_bass_kernel_spmd

F32 = mybir.dt.float32
F32R = mybir.dt.float32r
F16 = mybir.dt.float16

B, C, H, W = 2, 16, 128, 128
PS = (3, 5, 7)
PADS = (1, 2, 3)
NBG, NTG = 64, 8
NCORES = 8
NBANDS = 4
BH = H // NBANDS            # 32 out rows per core
RX = 38                     # x replica rows per core
CX = 134                    # x cols with halo (128 + 6)
RA = 38                     # max att rows (32 + 2*padmax)
RAL = 40                    # R sbuf rows (fold-tree halo)
RAS = [BH + 2 * p for p in PADS]   # att rows per branch: 34, 36, 38
SEQ = (2, 1, 0)             # branch processing order: p=7, 5, 3
W2BASE = [0, 2, 5]

_CACHE = {}


def _windows(ra):
    return [(r0, min(4, ra - r0)) for r0 in range(0, ra, 4)]


def _build_A():
    nc = bacc.Bacc("TRN2", target_bir_lowering=False, debug=False,
                   num_devices=NCORES)

    d_x8bg = nc.dram_tensor("x8bg", [112, RX, CX], F16, kind="ExternalInput")
    d_x8tg = nc.dram_tensor("x8tg", [112, RX, CX], F16, kind="ExternalInput")
    d_hug = nc.dram_tensor("hug", [3, RA, W], F32, kind="ExternalInput")
    d_rdiv = nc.dram_tensor("rdiv", [96, BH, W], F16, kind="ExternalInput")
    d_ones = nc.dram_tensor("oneslhs", [73, 72], F32, kind="ExternalInput")
    d_w1 = {}
    for s, M in (("bg", NBG), ("tg", NTG)):
        for n, p in enumerate(PS):
            d_w1[(s, n)] = nc.dram_tensor(
                f"w1{s}{n}", [16 * p, p, M], F16, kind="ExternalInput")
    d_w2bg = nc.dram_tensor("w2bg", [128, 9, 128], F16, kind="ExternalInput")
    d_w2tg = nc.dram_tensor("w2tg", [64, 3, 128], F16, kind="ExternalInput")
    d_fdiv = nc.dram_tensor("fdiv_out", [96, BH, W], F16,
                            kind="ExternalOutput")
    d_pool = nc.dram_tensor("pool_out", [32], F32, kind="ExternalOutput")

    with tile.TileContext(nc) as tc, ExitStack() as ctx:
        P = ctx.enter_context(tc.tile_pool(name="persist", bufs=1))
        pE = ctx.enter_context(tc.tile_pool(name="epool", bufs=3))
        pEtg = ctx.enter_context(tc.tile_pool(name="etg", bufs=2))
        pRcp = ctx.enter_context(tc.tile_pool(name="rcp", bufs=2))
        pS = ctx.enter_context(tc.tile_pool(name="spool", bufs=2))
        pR = ctx.enter_context(tc.tile_pool(name="rsb", bufs=3))
        pT = ctx.enter_context(tc.tile_pool(name="foldtmp", bufs=1))
        ps_c1bg = ctx.enter_context(
            tc.tile_pool(name="pc1bg", bufs=2, space=bass.MemorySpace.PSUM))
        ps_c1tg = ctx.enter_context(
            tc.tile_pool(name="pc1tg", bufs=2, space=bass.MemorySpace.PSUM))
        ps_den = ctx.enter_context(
            tc.tile_pool(name="pden", bufs=2, space=bass.MemorySpace.PSUM))
        ps_c2 = ctx.enter_context(
            tc.tile_pool(name="pc2", bufs=2, space=bass.MemorySpace.PSUM))

        # ---------------- persistent loads (spread across queues) --------
        x8 = {}
        t = P.tile([112, RX, CX], F16, tag="x8bg")
        nc.sync.dma_start(t[:, 0:19, :], d_x8bg[:, 0:19, :])
        nc.sync.dma_start(t[:, 19:RX, :], d_x8bg[:, 19:RX, :])
        x8["bg"] = t
        t = P.tile([112, RX, CX], F16, tag="x8tg")
        nc.gpsimd.dma_start(t[:, 0:19, :], d_x8tg[:, 0:19, :])
        nc.gpsimd.dma_start(t[:, 19:RX, :], d_x8tg[:, 19:RX, :])
        x8["tg"] = t
        ones_l = P.tile([73, 72], F32R, tag="ones")
        nc.gpsimd.dma_start(ones_l[:], d_ones[:])
        w1 = {}
        for s, M in (("bg", NBG), ("tg", NTG)):
            for n, p in enumerate(PS):
                t = P.tile([16 * p, p, M], F16, tag=f"w1{s}{n}")
                nc.scalar.dma_start(t[:], d_w1[(s, n)][:])
                w1[(s, n)] = t
        w2bg = P.tile([128, 9, 128], F16, tag="w2bg")
        nc.scalar.dma_start(w2bg[:], d_w2bg[:])
        w2tg = P.tile([64, 3, 128], F16, tag="w2tg")
        nc.scalar.dma_start(w2tg[:], d_w2tg[:])
        rdiv = P.tile([96, BH, W], F16, tag="rdiv")
        nc.scalar.dma_start(rdiv[:], d_rdiv[:])

        fdiv = P.tile([96, BH, W], F16, tag="fdiv")
        pacc = P.tile([96, 1], F32, tag="pacc")

        state = {}   # per-branch tiles: E chunks, S tiles, R tiles

        def conv1(k):
            n = SEQ[k]
            p, pad, ra = PS[n], PADS[n], RAS[n]
            rxo = 6 - 2 * pad
            wins = _windows(ra)
            Sbg = pS.tile([128, RA, 136], F16, tag="Sbg")
            Stg = pS.tile([72, RA, 144], F16, tag="Stg")
            nc.gpsimd.memset(Sbg[:, :, 0:4], 0.0)
            nc.gpsimd.memset(Sbg[:, :, 131:136], 0.0)
            nc.gpsimd.memset(Stg[:, :, 0:15], 0.0)
            nc.gpsimd.memset(Stg[:, :, 136:144], 0.0)
            st_e = {}

            def denom_block(w):
                r0, rr = wins[w]
                E = st_e[w]
                den = ps_den.tile([72, 4, W], F32, tag="den")
                nc.tensor.matmul(den[0:72, 0:rr, :], ones_l[:, 0:72],
                                 E[:, 0:rr, :], start=True, stop=True)
                rcp = pRcp.tile([72, 4, W], F32, tag="rcp")
                nc.vector.reciprocal_approx_fast(rcp[0:72, 0:rr, :],
                                                 den[0:72, 0:rr, :])
                nc.vector.tensor_mul(Sbg[0:64, r0:r0 + rr, 3:131],
                                     E[0:64, 0:rr, :], rcp[0:64, 0:rr, :])
                nc.gpsimd.tensor_mul(Stg[64:72, r0:r0 + rr, 8:136],
                                     E[64:72, 0:rr, :], rcp[64:72, 0:rr, :])
                # replica DMAs at half boundaries
                hend = wins[w][0] + wins[w][1]
                if hend == 20 or w == len(wins) - 1:
                    h0 = 0 if hend == 20 else 20
                    nc.sync.dma_start(Sbg[64:128, h0:hend, 4:132],
                                      Sbg[0:64, h0:hend, 3:131])
                    for g in range(p):
                        nc.sync.dma_start(Stg[8 * g:8 * g + 8, h0:hend,
                                              8 + g:136 + g],
                                          Stg[64:72, h0:hend, 8:136])

            for w, (r0, rr) in enumerate(wins):
                # conv1 bg window
                st = ps_c1bg.tile([64, 4, W], F32, tag="c1bg")
                for j in range(p):
                    nc.tensor.matmul(
                        st[0:64, 0:rr, :],
                        w1[("bg", n)][:, j, 0:64],
                        x8["bg"][0:16 * p, r0 + rxo:r0 + rxo + rr,
                                 j + 3 - pad:j + 3 - pad + W],
                        start=(j == 0), stop=(j == p - 1))
                E = pE.tile([73, 4, W], F32R, tag="E")
                st_e[w] = E
                nc.scalar.activation(E[0:64, 0:rr, :], st[0:64, 0:rr, :],
                                     mybir.ActivationFunctionType.Exp)
                nc.gpsimd.dma_start(E[72:73, 0:rr, :],
                                    d_hug[n:n + 1, r0:r0 + rr, :])
                # conv1 tg window
                stg = ps_c1tg.tile([8, 4, W], F32, tag="c1tg")
                for j in range(p):
                    nc.tensor.matmul(
                        stg[0:8, 0:rr, :],
                        w1[("tg", n)][:, j, :],
                        x8["tg"][0:16 * p, r0 + rxo:r0 + rxo + rr,
                                 j + 3 - pad:j + 3 - pad + W],
                        start=(j == 0), stop=(j == p - 1))
                etg = pEtg.tile([8, 4, W], F32R, tag="etg")
                nc.scalar.activation(etg[0:8, 0:rr, :], stg[0:8, 0:rr, :],
                                     mybir.ActivationFunctionType.Exp)
                nc.gpsimd.dma_start(E[64:72, 0:rr, :], etg[0:8, 0:rr, :])
                if w >= 1:
                    denom_block(w - 1)
            denom_block(len(wins) - 1)
            state[k] = (Sbg, Stg)

        def conv2(k):
            n = SEQ[k]
            p, pad, ra = PS[n], PADS[n], RAS[n]
            Sbg, Stg = state[k]
            Rbg = pR.tile([128, RAL, W], F16, tag="R")
            Rtg = pR.tile([128, RAL, W], F16, tag="R")
            nc.gpsimd.memset(Rbg[:, ra:RAL, :], 0.0)
            nc.gpsimd.memset(Rtg[:, ra:RAL, :], 0.0)
            nchk = (p + 1) // 2
            for r0, rr in _windows(ra):
                rp = ps_c2.tile([128, 4, W], F32, tag="c2")
                for ci in range(nchk):
                    jj = 2 * ci
                    nc.tensor.matmul(
                        rp[:, 0:rr, :],
                        w2bg[:, W2BASE[n] + ci, :],
                        Sbg[:, r0:r0 + rr, 3 + pad - jj:3 + pad - jj + W],
                        start=(ci == 0), stop=(ci == nchk - 1))
                nc.scalar.activation(Rbg[:, r0:r0 + rr, :], rp[:, 0:rr, :],
                                     mybir.ActivationFunctionType.Copy)
                rp2 = ps_c2.tile([128, 4, W], F32, tag="c2")
                nc.tensor.matmul(rp2[0:128, 0:rr, :],
                                 w2tg[0:8 * p, n, :],
                                 Stg[0:8 * p, r0:r0 + rr, 8 + pad:8 + pad + W],
                                 start=True, stop=True)
                nc.vector.tensor_copy(Rtg[:, r0:r0 + rr, :], rp2[:, 0:rr, :])
            state[k] = (Rbg, Rtg)

        def fold(k):
            n = SEQ[k]
            p = PS[n]
            Rbg, Rtg = state[k]
            for si, Rsb in ((0, Rbg), (1, Rtg)):
                if p == 3:
                    T2 = pT.tile([32, 33, W], F16, tag="T2")
                    nc.sync.dma_start(T2[0:32, 0:33, :], Rsb[32:64, 2:35, :])
                    nc.vector.tensor_add(Rsb[0:32, 0:33, :],
                                         Rsb[0:32, 0:33, :], T2[:])
                else:
                    T1 = pT.tile([64, 35, W], F16, tag="T1")
                    nc.sync.dma_start(T1[0:64, 0:35, :], Rsb[64:128, 4:39, :])
                    nc.vector.tensor_add(Rsb[0:64, 0:35, :],
                                         Rsb[0:64, 0:35, :], T1[:])
                    T2 = pT.tile([32, 33, W], F16, tag="T2")
                    nc.sync.dma_start(T2[0:32, 0:33, :], Rsb[32:64, 2:35, :])
                    nc.vector.tensor_add(Rsb[0:32, 0:33, :],
                                         Rsb[0:32, 0:33, :], T2[:])
                T3 = pT.tile([16, BH, W], F16, tag="T3")
                nc.sync.dma_start(T3[0:16, 0:BH, :], Rsb[16:32, 1:BH + 1, :])
                ftmp = pT.tile([16, BH, W], F16, tag="ftmp")
                nc.vector.tensor_add(ftmp[:], Rsb[0:16, 0:BH, :], T3[:])
                nc.gpsimd.dma_start(
                    fdiv[32 * n + 16 * si:32 * n + 16 * si + 16, :, :],
                    ftmp[:])
            nc.vector.scalar_tensor_tensor(
                fdiv[32 * n:32 * n + 32, :, :],
                fdiv[32 * n:32 * n + 32, :, :], 0.0,
                rdiv[32 * n:32 * n + 32, :, :],
                op0=mybir.AluOpType.bypass, op1=mybir.AluOpType.mult,
                accum_out=pacc[32 * n:32 * n + 32, :])
            nc.sync.dma_start(d_fdiv[32 * n:32 * n + 32, :, :],
                              fdiv[32 * n:32 * n + 32, :, :])

        # ---------------- pipelined emission ----------------
        conv1(0)
        conv1(1)
        conv2(0)
        fold(0)
        conv1(2)
        conv2(1)
        fold(1)
        conv2(2)
        fold(2)

        # pooled partial combine -> pool_out
        pb = P.tile([32, 1], F32, tag="pb")
        pc = P.tile([32, 1], F32, tag="pc")
        nc.sync.dma_start(pb[:], pacc[32:64, :])
        nc.sync.dma_start(pc[:], pacc[64:96, :])
        pool32a = P.tile([32, 1], F32, tag="pool32a")
        pool32 = P.tile([32, 1], F32, tag="pool32")
        nc.vector.tensor_add(pool32a[:], pacc[0:32, :], pb[:])
        nc.vector.tensor_add(pool32[:], pool32a[:], pc[:])
        nc.sync.dma_start(d_pool[:], pool32[:, 0])

    nc.compile()
    return nc


def _build_B():
    nc = bacc.Bacc("TRN2", target_bir_lowering=False, debug=False,
                   num_devices=NCORES)
    d_f = nc.dram_tensor("fdiv_in", [96, BH, W], F16, kind="ExternalInput")
    d_wt = nc.dram_tensor("wt96", [96, 1], F32, kind="ExternalInput")
    d_obg = nc.dram_tensor("out_bg", [C, BH, W], F16, kind="ExternalOutput")
    d_otg = nc.dram_tensor("out_tg", [C, BH, W], F16, kind="ExternalOutput")

    with tile.TileContext(nc) as tc, ExitStack() as ctx:
        Q = ctx.enter_context(tc.tile_pool(name="q", bufs=1))
        fdv = Q.tile([96, BH, W], F16, tag="fdv")
        wt = Q.tile([96, 1], F32, tag="wt")
        nc.sync.dma_start(fdv[0:48, :, :], d_f[0:48, :, :])
        nc.scalar.dma_start(fdv[48:96, :, :], d_f[48:96, :, :])
        nc.sync.dma_start(wt[:], d_wt[:])
        nc.vector.tensor_scalar_mul(fdv[:], fdv[:], wt[:])
        gb = Q.tile([32, BH, W], F16, tag="gb")
        gc = Q.tile([32, BH, W], F16, tag="gc")
        nc.sync.dma_start(gb[:], fdv[32:64, :, :])
        nc.scalar.dma_start(gc[:], fdv[64:96, :, :])
        nc.vector.tensor_add(fdv[0:32, :, :], fdv[0:32, :, :], gb[:])
        nc.vector.tensor_add(fdv[0:32, :, :], fdv[0:32, :, :], gc[:])
        nc.sync.dma_start(d_obg[:], fdv[0:16, :, :])
        nc.sync.dma_start(d_otg[:], fdv[16:32, :, :])

    nc.compile()
    return nc


# ======================= host-side prep =======================

def _prep_core(inputs, b, k):
    y0 = BH * k
    m = {}
    for s, key in (("bg", "bg"), ("tg", "tg")):
        x = np.asarray(inputs[key])[b]          # [C, H, W]
        x8 = np.zeros((7, C, RX, CX), np.float32)
        for g in range(7):
            lo = y0 - 6 + g
            hi = lo + RX
            slo, shi = max(lo, 0), min(hi, H)
            if slo < shi:
                x8[g, :, slo - lo:shi - lo, 3:131] = x[:, slo:shi, :]
        m[f"x8{s}"] = x8.reshape(112, RX, CX).astype(np.float16)

    hug = np.zeros((3, RA, W), np.float32)
    for n, pad in enumerate(PADS):
        for r in range(RA):
            y = y0 - pad + r
            if not (0 <= y < H):
                hug[n, r, :] = 1e30
    m["hug"] = hug

    rdiv = np.zeros((96, BH, W), np.float32)
    for n, pad in enumerate(PADS):
        yy = np.arange(H)
        rc = np.minimum(yy, pad) + np.minimum(H - 1 - yy, pad) + 1.0
        cc = np.minimum(yy[:W], pad) + np.minimum(W - 1 - yy[:W], pad) + 1.0
        div = np.outer(rc[y0:y0 + BH], cc) + 1e-8
        r = (1.0 / div).astype(np.float32)
        for si in range(2):
            base = 32 * n + 16 * si
            rdiv[base:base + 16] = r[None, :, :]
    m["rdiv"] = rdiv.astype(np.float16)

    ones = np.zeros((73, 72), np.float32)
    ones[0:64, 0:64] = 1.0
    ones[64:72, 64:72] = 1.0
    ones[72, :] = 1.0
    m["oneslhs"] = ones

    for s, M, nmem in (("bg", NBG, "bg_mem"), ("tg", NTG, "tg_mem")):
        for n, p in enumerate(PS):
            mem = np.asarray(inputs[f"{nmem}{n}"])          # [M, C*p*p]
            temp = float(np.asarray(inputs[f"{s}_temp{n}"])[0])
            D = C * p * p
            arr = mem.reshape(M, C, p, p)
            w1 = arr.transpose(2, 1, 3, 0).reshape(p * C, p, M).copy()
            m[f"w1{s}{n}"] = (w1 * (temp / np.sqrt(D))).astype(np.float16)

    # fold consumes group q at row shift +q where q = 2*pad - i
    w2bg = np.zeros((2, NBG, 9, 8, 16), np.float32)
    for n, p in enumerate(PS):
        pad = PADS[n]
        arr = np.asarray(inputs[f"bg_mem{n}"]).reshape(NBG, C, p, p)
        for ci in range((p + 1) // 2):
            for g in range(2):
                j = 2 * ci + g
                if j < p:
                    for i in range(p):
                        w2bg[g, :, W2BASE[n] + ci, 2 * pad - i, :] = \
                            arr[:, :, i, j]
    m["w2bg"] = w2bg.reshape(128, 9, 128).astype(np.float16)

    w2tg = np.zeros((8, NTG, 3, 8, 16), np.float32)
    for n, p in enumerate(PS):
        pad = PADS[n]
        arr = np.asarray(inputs[f"tg_mem{n}"]).reshape(NTG, C, p, p)
        for g in range(p):
            for i in range(p):
                w2tg[g, :, n, 2 * pad - i, :] = arr[:, :, i, g]
    m["w2tg"] = w2tg.reshape(64, 3, 128).astype(np.float16)
    return m


def _host_mlp(inputs, poolsum):
    """Per batch: pooled -> relu MLP -> softmax over scales -> wt96."""
    wt96 = np.zeros((96, 1), np.float32)
    for si, s in enumerate(("bg", "tg")):
        pooled = poolsum[16 * si:16 * si + 16] / (H * W)
        w1 = np.asarray(inputs[f"{s}_fc1_w"], np.float64)
        b1 = np.asarray(inputs[f"{s}_fc1_b"], np.float64)
        w2 = np.asarray(inputs[f"{s}_fc2_w"], np.float64)
        b2 = np.asarray(inputs[f"{s}_fc2_b"], np.float64)
        hdn = np.maximum(w1 @ pooled + b1, 0.0)
        logits = (w2 @ hdn + b2).reshape(3, 16)
        e = np.exp(logits - logits.max(axis=0, keepdims=True))
        wt = e / e.sum(axis=0, keepdims=True)
        for n in range(3):
            wt96[32 * n + 16 * si:32 * n + 16 * si + 16, 0] = wt[n]
    return wt96


def kernel(**inputs):
    if "ncA" not in _CACHE:
        _CACHE["ncA"] = _build_A()
        _CACHE["ncB"] = _build_B()

    in_maps = []
    for core in range(NCORES):
        b, k = divmod(core, NBANDS)
        in_maps.append(_prep_core(inputs, b, k))

    resA = run_bass_kernel_spmd(_CACHE["ncA"], in_maps, list(range(NCORES)))

    # host glue: reduce pooled partials within each batch's 4-band group,
    # then the tiny fusion MLP (exact, fp64)
    wt = {}
    for b in range(B):
        poolsum = np.sum([resA.results[b * NBANDS + k]["pool_out"]
                          for k in range(NBANDS)], axis=0).astype(np.float64)
        wt[b] = _host_mlp(inputs, poolsum)

    in_mapsB = []
    for core in range(NCORES):
        b, k = divmod(core, NBANDS)
        in_mapsB.append({
            "fdiv_in": resA.results[core]["fdiv_out"],
            "wt96": wt[b],
        })
    resB = run_bass_kernel_spmd(_CACHE["ncB"], in_mapsB, list(range(NCORES)))

    f_bc = np.zeros((B, C, H, W), np.float32)
    f_tg = np.zeros((B, C, H, W), np.float32)
    for core in range(NCORES):
        b, k = divmod(core, NBANDS)
        y0 = BH * k
        f_bc[b, :, y0:y0 + BH, :] = resB.results[core]["out_bg"].astype(np.float32)
        f_tg[b, :, y0:y0 + BH, :] = resB.results[core]["out_tg"].astype(np.float32)
    return (f_bc, f_tg)
